# revision 2
# baseline (speedup 1.0000x reference)
"""Trainium2 Bass kernel for nn_Meta_Graph1_40114994545303 (gnn_message_passing).

Math: the reference returns only the global-node row of the GCN output.
With mask = (attribute_label > 0), star adjacency means
    out[s, :] = tanh( (sum_a mask[s,a] * attribute_feat[s,a,:]) @ W + b )
and x never reaches the output (adj[A, A] = 0).

Strategy: data-parallel over batch, 32 samples per core on 8 cores.
Per core:
  stage 1: masked sum over attributes as a block-diagonal matmul
           (feat streamed as the moving operand, mask block-diag stationary).
           The mask values carry the W-dequant scale s, so stage-1 output is
           s*masked_sum and no constant is baked into the NEFF.
  transpose the [32, 2048] scaled masked sum to [2048, 32] via DVE 32x32 blocks
  stage 2: [32, 2048] @ W_q as 16 K-chunk matmuls. W is stored int8 in HBM
           (quantized host-side, clip 3.9 sigma) and cast to fp16 during the
           SWDGE DMA; integer values +-127 are exact in fp16, so the matmul
           computes s*ms @ W_q = ms @ W exactly up to int8 quantization.
           Bias folded in as a rank-1 (K=1) matmul of ones x b into the same
           PSUM accumulation; tanh on the scalar engine, DMA out.

feat and W stream on the single gpsimd SWDGE queue in that order so the
feat bytes (needed first) are never starved by W packets.
"""

import os
from contextlib import ExitStack

import numpy as np

import concourse.bacc as bacc
import concourse.mybir as mybir

B, A, D = 256, 32, 2048
NCORES = 8
S = B // NCORES  # 32 samples per core
P = 128
KC1 = (S * A) // P  # 8 k-chunks in stage 1 (contraction over (sample, attr))
KC2 = D // P  # 16 k-chunks in stage 2 (contraction over d_in)
NT = D // 512  # 4 psum-bank-wide column tiles
F32 = mybir.dt.float32
F16 = mybir.dt.float16
I8 = mybir.dt.int8

W_CLIP_SIGMAS = 3.9


def build_nc_int8w():
    """W stored int8 in HBM, cast->fp16 during SWDGE DMA. feat fp16."""
    cdt = F16
    cf = 4  # feat k-chunks per DMA (2 transfers of 2 MB)
    WCH = [4, 4, 4, 2, 2]  # W transfer sizes in k2-chunks; small tail chunks
    WST = [0, 4, 8, 12, 14]
    NF, NW = KC1 // cf, len(WCH)
    nc = bacc.Bacc("TRN2", target_bir_lowering=False, debug=False)

    feat = nc.dram_tensor("feat", [S * A, D], cdt, kind="ExternalInput")
    mbdt = nc.dram_tensor("mbdt", [P, KC1 * S], cdt, kind="ExternalInput")
    w8 = nc.dram_tensor("w8", [D, D], I8, kind="ExternalInput")
    bias = nc.dram_tensor("bias", [1, D], cdt, kind="ExternalInput")
    onesd = nc.dram_tensor("ones", [1, S], cdt, kind="ExternalInput")
    out = nc.dram_tensor("out", [S, D], F32, kind="ExternalOutput")

    with ExitStack() as ctx:
        feat_sb = ctx.enter_context(nc.sbuf_tensor([P, KC1, D], cdt))
        w_sb = ctx.enter_context(nc.sbuf_tensor([P, KC2, D], cdt))
        mbdt_sb = ctx.enter_context(nc.sbuf_tensor([P, KC1, S], cdt))
        bias_sb = ctx.enter_context(nc.sbuf_tensor([1, D], cdt))
        ones_sb = ctx.enter_context(nc.sbuf_tensor([1, S], cdt))
        msc_sb = ctx.enter_context(nc.sbuf_tensor([P, 512], cdt))
        msT_sb = ctx.enter_context(nc.sbuf_tensor([P, KC2, S], cdt))
        out_sb = ctx.enter_context(nc.sbuf_tensor([P, 512], F32))
        pm_bank = ctx.enter_context(nc.psum_tensor([P, 512], F32))
        po_bank = ctx.enter_context(nc.psum_tensor([P, 512], F32))
        fsems = [ctx.enter_context(nc.semaphore(f"fs{g}")) for g in range(NF)]
        wsems = [ctx.enter_context(nc.semaphore(f"ws{g}")) for g in range(NW)]
        csem = ctx.enter_context(nc.semaphore("csem"))
        osem = ctx.enter_context(nc.semaphore("osem"))
        s1_sem = ctx.enter_context(nc.semaphore("s1_sem"))
        tr_sem = ctx.enter_context(nc.semaphore("tr_sem"))
        s2_sem = ctx.enter_context(nc.semaphore("s2_sem"))
        act_sem = ctx.enter_context(nc.semaphore("act_sem"))
        osem2 = ctx.enter_context(nc.semaphore("osem2"))
        block = ctx.enter_context(nc.Block(no_gpsimd_drain=True))

        @block.gpsimd
        def _(gpsimd):
            # feat first, then W, on the single SWDGE ring: FIFO order
            # guarantees feat bytes land before W competes for bandwidth.
            for g in range(NF):
                gpsimd.dma_start(
                    feat_sb[:, g * cf : (g + 1) * cf, :],
                    feat[g * cf * P : (g + 1) * cf * P, :].rearrange(
                        "(c p) d -> p c d", p=P
                    ),
                ).then_inc(fsems[g], 16)
            for g in range(NW):
                st, ln = WST[g], WCH[g]
                gpsimd.dma_start(
                    w_sb[:, st : st + ln, :],
                    w8[st * P : (st + ln) * P, :].rearrange(
                        "(c p) d -> p c d", p=P
                    ),
                ).then_inc(wsems[g], 16)

        @block.sync
        def _(sync):
            sync.wait_ge(act_sem, 1)
            for n in (0, 2):
                sync.dma_start(
                    out[:, n * 512 : (n + 1) * 512], out_sb[n * S : (n + 1) * S, :]
                ).then_inc(osem2, 16)
            sync.wait_ge(osem2, 32)

        @block.scalar
        def _(scalar):
            scalar.dma_start(
                mbdt_sb[:], mbdt[:].rearrange("p (k j) -> p k j", k=KC1)
            ).then_inc(csem, 16)
            scalar.dma_start(bias_sb[:], bias[:]).then_inc(csem, 16)
            scalar.dma_start(ones_sb[:], onesd[:]).then_inc(csem, 16)
            scalar.wait_ge(s2_sem, 1)
            nc.scalar.activation(
                out_sb[:], po_bank[:], mybir.ActivationFunctionType.Tanh
            ).then_inc(act_sem, 1)
            scalar.wait_ge(act_sem, 1)
            for n in (1, 3):
                scalar.dma_start(
                    out[:, n * 512 : (n + 1) * 512], out_sb[n * S : (n + 1) * S, :]
                ).then_inc(osem, 16)
            scalar.wait_ge(osem, 32)

        @block.vector
        def _(vector):
            vector.wait_ge(s1_sem, 1)
            nc.vector.tensor_copy(msc_sb[:], pm_bank[:])
            nc.vector.drain()
            last = None
            for n in range(NT):
                for q in range(512 // 32):
                    d0 = n * 512 + q * 32
                    k2, r = divmod(d0, P)
                    last = nc.vector.transpose(
                        msT_sb[r : r + 32, k2, :],
                        msc_sb[n * S : (n + 1) * S, q * 32 : (q + 1) * 32],
                    )
            last.then_inc(tr_sem, 1)

        @block.tensor
        def _(tensor):
            tensor.wait_ge(csem, 48)  # mbdt/bias/ones resident
            # bias as the FIRST accumulation into po_bank (off the tail path)
            for n in range(NT):
                nc.tensor.matmul(
                    po_bank[n * S : (n + 1) * S, :],
                    ones_sb[:],
                    bias_sb[:, n * 512 : (n + 1) * 512],
                    start=True,
                    stop=False,
                    tile_position=(0, n * S),
                    skip_group_check=True,
                )
            last = None
            for g in range(NF):
                tensor.wait_ge(fsems[g], 16)
                for c in range(cf):
                    k = g * cf + c
                    for n in range(NT):
                        last = nc.tensor.matmul(
                            pm_bank[n * S : (n + 1) * S, :],
                            mbdt_sb[:, k, :],
                            feat_sb[:, k, n * 512 : (n + 1) * 512],
                            start=(k == 0),
                            stop=(k == KC1 - 1),
                            tile_position=(0, n * S),
                            skip_group_check=True,
                        )
            last.then_inc(s1_sem, 1)
            tensor.wait_ge(tr_sem, 1)
            lastb = None
            for g in range(NW):
                tensor.wait_ge(wsems[g], 16)
                for c in range(WCH[g]):
                    k2 = WST[g] + c
                    for n in range(NT):
                        lastb = nc.tensor.matmul(
                            po_bank[n * S : (n + 1) * S, :],
                            msT_sb[:, k2, :],
                            w_sb[:, k2, n * 512 : (n + 1) * 512],
                            start=False,
                            stop=(k2 == KC2 - 1),
                            tile_position=(0, n * S),
                            skip_group_check=True,
                        )
            lastb.then_inc(s2_sem, 1)

    nc.compile()
    return nc


def _host_prep_int8w(inputs: dict):
    feat = np.asarray(inputs["attribute_feat"], dtype=np.float32)
    label = np.asarray(inputs["attribute_label"])
    w = np.asarray(inputs["W"], dtype=np.float32)
    b = np.asarray(inputs["b"], dtype=np.float32).reshape(1, D)
    mask = (label > 0).astype(np.float32)

    wstd = float(w.std())
    s = (W_CLIP_SIGMAS * wstd / 127.0) if wstd > 0 else 1.0
    w_q = np.clip(np.round(w / s), -127, 127).astype(np.int8)

    in_maps = []
    for c in range(NCORES):
        feat_c = feat[c * S : (c + 1) * S].reshape(S * A, D).astype(np.float16)
        m_c = mask[c * S : (c + 1) * S] * s  # fold dequant scale into the mask
        mbd = np.zeros((KC1, P, S), np.float32)
        for k in range(KC1):
            for sl in range(P // A):  # 4 samples per 128-row chunk
                smp = (P // A) * k + sl
                mbd[k, sl * A : (sl + 1) * A, smp] = m_c[smp]
        mbd_dev = np.ascontiguousarray(mbd.transpose(1, 0, 2)).reshape(P, KC1 * S)
        in_maps.append(
            {
                "feat": feat_c,
                "mbdt": mbd_dev.astype(np.float16),
                "w8": w_q,
                "bias": b.astype(np.float16),
                "ones": np.ones((1, S), np.float16),
            }
        )
    return in_maps


_NC_CACHE: dict = {}


def run(inputs: dict, compute_dtype: str = "fp16", trace: bool = False):
    from concourse.bass_utils import run_bass_kernel_spmd

    if "int8w" not in _NC_CACHE:
        _NC_CACHE["int8w"] = build_nc_int8w()
    nc = _NC_CACHE["int8w"]
    in_maps = _host_prep_int8w(inputs)
    res = run_bass_kernel_spmd(nc, in_maps, list(range(NCORES)), trace=trace)
    out = np.concatenate([res.results[c]["out"] for c in range(NCORES)], axis=0)
    return out, res


def kernel(**inputs) -> np.ndarray:
    out, _ = run(inputs)
    return out
